# revision 19
# baseline (speedup 1.0000x reference)
"""DeepGMM Trainium2 kernel — batch-parallel over 8 NeuronCores.

Math: out[b,m,k] = w_mk * (-0.5*(quad + D*log2pi) - logdet_mk), where
quad = ||L^-1 (f_b - mu)||^2, f = relu(x@W+b).
Let A' = sqrt(0.5 w) L^-1, c' = A' mu, z = A' f.
0.5 w quad = ||z - c'||^2 = S1 + yneg + kappa,
  S1 = sum z^2, yneg = <f, h'> (h' = -2 A'^T c'), kappa = ||c'||^2.
out = gamma - (S1 + yneg), gamma = beta - kappa,
  beta = -0.5 w D log2pi - w logdet.

Sharding: data-parallel over batch. Each core takes 512 rows (4 blocks
of 128) and all 80 (m,k) pairs; outputs concatenate along batch.

Device schedule per core:
  feats GEMM (bf16) -> fT [128, 2, 512]
  yneg for all pairs via one small GEMM  f @ H   (H = all h' columns)
  waves of 8 pairs: z-GEMM with f stationary, 2 pairs per PSUM bank
    (A'^T upper-triangular: second K-half only touches cols 128:256)
  reduce Sum z^2: per wave, NA pairs on ACT (Square+accum) and the
    rest on DVE via one multi-group bn_stats instruction
    (Sum x^2 = cnt*var + cnt*mean^2, combined in a small postpass)
  out = gamma - (S1 + yneg) on DVE, DMA out.
"""
import sys
import types

sys.path.insert(0, "/opt/trn_rl_repo")


def _install_ntff_shim():
    # The axon boot looks for antenv.axon_hooks to register its NTFF
    # profiling hook; if the module is missing, provide a placeholder.
    if "antenv.axon_hooks" in sys.modules:
        return
    try:
        import antenv.axon_hooks  # noqa: F401
        return
    except ImportError:
        pass
    mod = types.ModuleType("antenv.axon_hooks")
    holder = [None]
    mod.set_axon_ntff_profile_hook = lambda h: holder.__setitem__(0, h)
    mod.get_axon_ntff_profile_hook = lambda: holder[0]
    sys.modules["antenv.axon_hooks"] = mod
    try:
        import antenv
        antenv.axon_hooks = mod
    except ImportError:
        pass


_install_ntff_shim()

import numpy as np
import ml_dtypes

B, D_IN, D_F = 4096, 1024, 256
M, K = 10, 8
NPAIR = 80
NCORE = 8
BC = B // NCORE        # 512 batch rows per core
NB = BC // 128         # 4 b-blocks
NPP = NPAIR // 2       # 40 pair-pairs (2 pairs share a PSUM bank)
WPAIR = 8              # pairs per wave (one [128,2048] psum tile)
WAVES = NPAIR // WPAIR # 10 waves per b-block
NA = 2                 # pairs per wave reduced on ACT (rest on DVE)
ND = WPAIR - NA
RHS_CH = 8             # rhs DMA chunks
LOG2PI = float(np.log(2.0 * np.pi))
BF16 = ml_dtypes.bfloat16

_cache = {}


def _build_module():
    import concourse.bass as bass
    import concourse.tile as tile
    import concourse.mybir as mybir
    from concourse import bacc

    dt = mybir.dt
    AF = mybir.ActivationFunctionType
    ALU = mybir.AluOpType

    nc = bacc.Bacc("TRN2", target_bir_lowering=False, debug=False,
                   enable_asserts=False, num_devices=NCORE)

    x_d = nc.dram_tensor("x_in", [128, 8, BC], dt.bfloat16,
                         kind="ExternalInput").ap()
    w_d = nc.dram_tensor("w_in", [128, 8, D_F], dt.bfloat16,
                         kind="ExternalInput").ap()
    b_d = nc.dram_tensor("b_in", [128, 2], dt.float32,
                         kind="ExternalInput").ap()
    # rhs columns of the two pairs sharing a PSUM bank are interleaved
    # (pair A on even, pair B on odd cols) so bn_stats' even/odd stats
    # separate them.  lo: K-rows 0:128, all 256 cols per pair.
    # hi: K-rows 128:256, cols 128:256 per pair (A'^T upper-triangular).
    rhs_lo_d = nc.dram_tensor("rhs_lo", [128, NPP, 512], dt.bfloat16,
                              kind="ExternalInput").ap()
    rhs_hi_d = nc.dram_tensor("rhs_hi", [128, NPP, 256], dt.bfloat16,
                              kind="ExternalInput").ap()
    h_d = nc.dram_tensor("h_in", [128, 2, NPAIR], dt.bfloat16,
                         kind="ExternalInput").ap()
    gam_d = nc.dram_tensor("gam_in", [128, NPAIR], dt.float32,
                           kind="ExternalInput").ap()
    out_d = nc.dram_tensor("out", [BC, NPAIR], dt.float32,
                           kind="ExternalOutput").ap()

    with tile.TileContext(nc) as tc:
        with (
            tc.tile_pool(name="const", bufs=1) as constp,
            tc.tile_pool(name="feat", bufs=1) as fp,
            tc.tile_pool(name="junk", bufs=4) as jp,
            tc.tile_pool(name="stat", bufs=2) as stp,
            tc.tile_pool(name="s1", bufs=2) as s1p,
            tc.tile_pool(name="pp", bufs=8, space="PSUM") as pp,
            tc.tile_pool(name="outp", bufs=4) as op,
        ):
            # ---- input DMAs (chunked; ordered by first use) ----
            x_sb = constp.tile([128, 8, BC], dt.bfloat16)
            w_sb = constp.tile([128, 8, D_F], dt.bfloat16)
            for kb in range(8):
                nc.sync.dma_start(w_sb[:, kb], w_d[:, kb])
                nc.sync.dma_start(x_sb[:, kb], x_d[:, kb])
            b_sb = constp.tile([128, 2], dt.float32)
            nc.sync.dma_start(b_sb[:], b_d[:])
            rhs_lo = constp.tile([128, NPP, 512], dt.bfloat16)
            rhs_hi = constp.tile([128, NPP, 256], dt.bfloat16)
            chw = NPP // RHS_CH
            nc.sync.dma_start(rhs_lo[:, 0:chw], rhs_lo_d[:, 0:chw])
            nc.sync.dma_start(rhs_hi[:, 0:chw], rhs_hi_d[:, 0:chw])
            h_sb = constp.tile([128, 2, NPAIR], dt.bfloat16)
            nc.sync.dma_start(h_sb[:], h_d[:])
            gam_sb = constp.tile([128, NPAIR], dt.float32)
            nc.sync.dma_start(gam_sb[:], gam_d[:])
            for j in range(1, RHS_CH):
                nc.sync.dma_start(rhs_lo[:, j * chw:(j + 1) * chw],
                                  rhs_lo_d[:, j * chw:(j + 1) * chw])
                nc.sync.dma_start(rhs_hi[:, j * chw:(j + 1) * chw],
                                  rhs_hi_d[:, j * chw:(j + 1) * chw])

            # warm the ACT function table while DMAs stream in
            warm = jp.tile([128, 1], dt.float32, tag="warm")
            nc.vector.memset(warm[:], 0.0)
            wj = jp.tile([128, 1], dt.float32, tag="warmo")
            nc.scalar.activation(wj[:], warm[:], AF.Relu)

            # ---- feats: fT [128, 2, BC] = relu(x@W+b)^T (bf16) ----
            ft = fp.tile([128, 2, BC], dt.bfloat16)
            psf0 = pp.tile([128, 512], dt.float32, tag="ps", name="psf0")
            psf1 = pp.tile([128, 512], dt.float32, tag="ps", name="psf1")
            for kb in range(8):
                for fb, psf in ((0, psf0), (1, psf1)):
                    nc.tensor.matmul(
                        psf[:], lhsT=w_sb[:, kb, fb * 128:(fb + 1) * 128],
                        rhs=x_sb[:, kb, :], start=(kb == 0), stop=(kb == 7),
                        skip_group_check=True)
            for fb, psf in ((0, psf0), (1, psf1)):
                nc.scalar.activation(ft[:, fb, :], psf[:], AF.Relu,
                                     bias=b_sb[:, fb:fb + 1])

            # ---- yneg for all pairs: y[b, p] = <f_b, h'_p> ----
            y_sb = fp.tile([128, NB, NPAIR], dt.float32)
            for bb in range(NB):
                l1 = ft[:, 0, bb * 128:(bb + 1) * 128]
                l2 = ft[:, 1, bb * 128:(bb + 1) * 128]
                pyf = pp.tile([128, 512], dt.float32, tag="ps", name="pyf")
                py = pyf[:, 0:NPAIR]
                nc.tensor.matmul(py[:], lhsT=l1, rhs=h_sb[:, 0, :],
                                 start=True, stop=False)
                nc.tensor.matmul(py[:], lhsT=l2, rhs=h_sb[:, 1, :],
                                 start=False, stop=True)
                nc.scalar.copy(y_sb[:, bb, :], py[:])

            # ---- z-GEMM + split reduce, one PSUM bank (2 pairs) at a time
            # bank i of 40 per b-block: i%3==0 -> ACT (14), else DVE (26)
            for bb in range(NB):
                l1 = ft[:, 0, bb * 128:(bb + 1) * 128]
                l2 = ft[:, 1, bb * 128:(bb + 1) * 128]
                s1 = s1p.tile([128, NPAIR], dt.float32, tag="s1")
                # stats[g, j] = bn_stats of DVE bank 3g+1+j
                # (6 vals: cnt/mean/cnt*var for even=pairA, odd=pairB)
                stats = stp.tile([128, 13, 2, 6], dt.float32, tag="st")
                for i in range(NPP):
                    pz = pp.tile([128, 512], dt.float32, tag="ps")
                    nc.tensor.matmul(pz[:], lhsT=l1, rhs=rhs_lo[:, i, :],
                                     start=True, stop=False)
                    # K-half 1 touches only cols 128:256 of each pair
                    # = interleaved bank cols 256:512 (contiguous)
                    nc.tensor.matmul(pz[:, 256:512], lhsT=l2,
                                     rhs=rhs_hi[:, i, :],
                                     start=False, stop=True,
                                     skip_group_check=True)
                    if i % 3 == 0:
                        # in-place square (PSUM->PSUM is ACT's fast port);
                        # only accum_out is consumed
                        bv = pz.rearrange("p (x two) -> p two x", two=2)
                        for e in range(2):
                            nc.scalar.activation(
                                bv[:, e], bv[:, e], AF.Square,
                                accum_out=s1[:, 2 * i + e:2 * i + e + 1])
                    else:
                        nc.vector.bn_stats(stats[:, (i - 1) // 3, (i - 1) % 3],
                                           pz[:])
                # postpass: S1 = cnt*var + cnt*mean^2, cnt = 256
                means = stats.rearrange(
                    "p g j (two three) -> p g j two three", three=3
                )[:, :, :, :, 1]
                m2 = jp.tile([128, 13, 2, 2], dt.float32, tag="m2")
                nc.gpsimd.tensor_tensor(m2[:], means, means, op=ALU.mult)
                varis = stats.rearrange(
                    "p g j (two three) -> p g j two three", three=3
                )[:, :, :, :, 2]
                s1dve = s1[:, 0:78].rearrange(
                    "p (g j e) -> p g j e", j=3, e=2)[:, :, 1:3, :]
                nc.vector.scalar_tensor_tensor(
                    s1dve, in0=m2[:], scalar=256.0, in1=varis,
                    op0=ALU.mult, op1=ALU.add)
                # combine: out = gamma - (S1 + yneg)
                tmp = op.tile([128, NPAIR], dt.float32, tag="tmp")
                nc.vector.tensor_tensor(tmp[:], s1[:], y_sb[:, bb, :],
                                        op=ALU.add)
                ot = op.tile([128, NPAIR], dt.float32, tag="ot")
                nc.vector.tensor_sub(ot[:], gam_sb[:], tmp[:])
                nc.sync.dma_start(out_d[bb * 128:(bb + 1) * 128, :], ot[:])
    nc.finalize()
    return nc


def _prep_inputs(x, W, b, means, covs, weights):
    # host: shard/cast/layout + small per-pair parameter preprocessing
    x = np.asarray(x, np.float32)
    W = np.asarray(W, np.float32)
    b = np.asarray(b, np.float32)
    means = np.asarray(means, np.float32).reshape(NPAIR, D_F)
    covs = np.asarray(covs, np.float32).reshape(NPAIR, D_F, D_F)
    weights = np.asarray(weights, np.float32)

    w_in = np.ascontiguousarray(
        W.reshape(8, 128, D_F).transpose(1, 0, 2)).astype(BF16)
    b_in = np.ascontiguousarray(b.reshape(2, 128).T).astype(np.float32)

    ew = np.exp(weights - weights.max(axis=1, keepdims=True))
    w_sm = (ew / ew.sum(axis=1, keepdims=True)).reshape(NPAIR)

    from scipy.linalg import solve_triangular
    rhs_lo = np.zeros((128, NPP, 512), np.float32)
    rhs_hi = np.zeros((128, NPP, 256), np.float32)
    h_in = np.zeros((128, 2, NPAIR), np.float32)
    gam = np.zeros(NPAIR, np.float32)
    eye = np.eye(D_F, dtype=np.float32)
    for q in range(NPAIR):
        L = np.tril(covs[q])
        A = solve_triangular(L, eye, lower=True)
        s = np.sqrt(0.5 * w_sm[q])
        Ap = s * A
        cp = Ap @ means[q]
        hp = -2.0 * (Ap.T @ cp)
        logdet = np.log(np.diag(L)).sum()
        beta = -0.5 * w_sm[q] * D_F * LOG2PI - w_sm[q] * logdet
        gam[q] = beta - float(cp @ cp)
        ApT = Ap.T  # [k, i]; upper-triangular (zero for k > i)
        pp_, e = q // 2, q % 2
        # bank col for feature i = 2*i + e (pairs interleaved)
        rhs_lo[:, pp_, e::2] = ApT[0:128, :]
        rhs_hi[:, pp_, e::2] = ApT[128:256, 128:256]
        h_in[:, 0, q] = hp[0:128]
        h_in[:, 1, q] = hp[128:256]
    rhs_lo = rhs_lo.astype(BF16)
    rhs_hi = rhs_hi.astype(BF16)
    h_in = h_in.astype(BF16)
    gam_in = np.ascontiguousarray(
        np.broadcast_to(gam[None, :], (128, NPAIR))).astype(np.float32)

    in_maps = []
    for c in range(NCORE):
        xc = x[c * BC:(c + 1) * BC]  # [BC, D_IN]
        x_in = np.ascontiguousarray(
            xc.T.reshape(8, 128, BC).transpose(1, 0, 2)).astype(BF16)
        in_maps.append({
            "x_in": x_in, "w_in": w_in, "b_in": b_in,
            "rhs_lo": rhs_lo, "rhs_hi": rhs_hi,
            "h_in": h_in, "gam_in": gam_in,
        })
    return in_maps


def kernel(x, W, b, means, covs, weights, _want_trace=False):
    from concourse import bass_utils

    if "nc" not in _cache:
        _cache["nc"] = _build_module()
    nc = _cache["nc"]
    in_maps = _prep_inputs(x, W, b, means, covs, weights)
    res = bass_utils.run_bass_kernel_spmd(
        nc, in_maps, core_ids=list(range(NCORE)), trace=_want_trace)
    if _want_trace:
        _cache["last_results"] = res
    out = np.concatenate([res.results[c]["out"] for c in range(NCORE)],
                         axis=0)
    return np.ascontiguousarray(out.reshape(B, M, K).astype(np.float32))
